# revision 4
# baseline (speedup 1.0000x reference)
"""DeepseekV3 MoE layer on 8 Trainium2 NeuronCores (expert-parallel).

Contract: kernel(**inputs) takes the FULL unsharded inputs and returns the
FULL output [4, 2048, 2048] f32.

Strategy:
  - Routing (sigmoid gate + group-limited top-6) computed on host in numpy.
  - Expert parallelism: 32 experts -> 8 cores x 4 slots. Experts are
    assigned to (core, slot) by sorted token count so every core runs an
    identical static program (slot capacities shared across cores).
  - Token dispatch on host: tokens gathered/padded per expert, transposed
    to feature-major [H, ncap] blocks; per-token routing weights applied
    on device; host scatter-adds the (disjoint) results.
  - Shared MLP data-parallel over tokens (1024 tokens per core).
  - Device kernel: fp32r matmuls (full PE rate, ~1.5e-4 matmul error),
    silu/mul on ACT/DVE, weights streamed from HBM once per token chunk.
"""
import sys
import os

sys.path.insert(0, "/opt/trn_rl_repo")

import numpy as np

import concourse.bacc as bacc_mod
import concourse.mybir as mybir
import concourse.tile as tile
from concourse.bass_utils import run_bass_kernel_spmd

F32 = mybir.dt.float32
F32R = mybir.dt.float32r
P = 128

# Problem constants (hardcoded per contract)
B, S, H = 4, 2048, 2048
T = B * S                      # 8192 tokens
E = 32                         # experts
TOPK = 6
N_GROUPS = 4
N_LIMITED = 2
MI = 1408                      # expert intermediate
SH = 2816                      # shared intermediate
NH = H // P                    # 16 h-tiles
NM = MI // P                   # 11 m-tiles (expert)
NMS = SH // P                  # 22 m-tiles (shared)
NCORES = 8
NSLOTS = 4
HC = 512                       # stage-2 output column chunk
NHC = H // HC                  # 4
TOK_SH = T // NCORES           # 1024 shared-MLP tokens per core


def _round_up(x, m):
    return ((x + m - 1) // m) * m


_GOOD_CHUNKS = (896, 768, 512, 384, 256)  # ck % 512 in {0, 256, 384}: no <256 pieces


def _chunks_of(cap):
    """Split a slot capacity (multiple of 128) into the fewest chunks from
    _GOOD_CHUNKS (fp32r needs moving pieces >=256 for full PE rate)."""
    best = {0: []}
    for c in range(128, cap + 1, 128):
        cands = []
        for g in _GOOD_CHUNKS:
            if g <= c and (c - g) in best:
                cands.append([g] + best[c - g])
        if c in (128,) or not cands:
            if (c - 128) in best:
                cands.append([128] + best[c - 128])   # last-resort tiny chunk
        if cands:
            best[c] = min(cands, key=lambda l: (len(l), l.count(128)))
    return sorted(best[cap], reverse=True)


def _pieces_of(ck):
    """Split a chunk into moving-dim pieces of <=512 (bank-aligned)."""
    out = []
    off = 0
    while off < ck:
        pl = min(512, ck - off)
        out.append((off, pl))
        off += pl
    return out


def _mlp_chunk(nc, wpool, iopool, respool, pspool, xt_dram, toff, ck,
               wg_dram, wu_dram, wd_dram, n_mt, rws, rw_base, y_dram):
    """One token chunk through gate/up/silu*up/down (+ routing-weight scale).

    xt_dram: [H, ncap-like] feature-major tokens (f32r dram)
    wg_dram/wu_dram: [n_mt, 128, H]   (m-tile, p, ko*128)
    wd_dram: [NHC, 128, n_mt*512]     (hc, p(m-row), mt*512)
    rws: resident [128, NT] routing-weight tile or None
    y_dram: [ntok, H] output rows
    """
    ACT = mybir.ActivationFunctionType
    # ---- load + round X^T chunk ----
    xtr = respool.tile([P, NH, ck], F32R, tag="xtr")
    for h in range(NH):
        nc.sync.dma_start(xtr[:, h], xt_dram[h * P:(h + 1) * P, toff:toff + ck])
    nc.vector.tensor_copy(out=xtr[:], in_=xtr[:])

    A = respool.tile([P, n_mt, ck], F32R, tag="A")
    pieces = _pieces_of(ck)
    # ---- stage 1: G = X@Wg, U = X@Wu, A = silu(G)*U  (feature-major) ----
    for m in range(n_mt):
        wgr = wpool.tile([P, NH, P], F32R, tag="wgr")
        wur = wpool.tile([P, NH, P], F32R, tag="wur")
        nc.sync.dma_start(wgr[:], wg_dram[m].rearrange("p (ko x) -> p ko x", x=P))
        nc.sync.dma_start(wur[:], wu_dram[m].rearrange("p (ko x) -> p ko x", x=P))
        nc.vector.tensor_copy(out=wgr[:], in_=wgr[:])
        nc.vector.tensor_copy(out=wur[:], in_=wur[:])
        for (po, pl) in pieces:
            g = pspool.tile([P, pl], F32, tag="g")
            u = pspool.tile([P, pl], F32, tag="u")
            for h in range(NH):
                nc.tensor.matmul(g[:], wgr[:, h], xtr[:, h, po:po + pl],
                                 start=(h == 0), stop=(h == NH - 1))
            for h in range(NH):
                nc.tensor.matmul(u[:], wur[:, h], xtr[:, h, po:po + pl],
                                 start=(h == 0), stop=(h == NH - 1))
            sg = iopool.tile([P, pl], F32, tag="sg")
            nc.scalar.activation(out=sg[:], in_=g[:], func=ACT.Silu)
            # fp32r output = the rounding producer for stage 2
            nc.vector.tensor_mul(out=A[:, m, po:po + pl], in0=sg[:], in1=u[:])

    # ---- stage 2: Y = A @ Wd, scaled by routing weight, token-major ----
    ntt = ck // P
    for hc in range(NHC):
        wdr = wpool.tile([P, n_mt, HC], F32R, tag="wdr")
        nc.sync.dma_start(wdr[:], wd_dram[hc].rearrange("p (mt x) -> p mt x", x=HC))
        nc.vector.tensor_copy(out=wdr[:], in_=wdr[:])
        for t in range(ntt):
            yp = pspool.tile([P, HC], F32, tag="y")
            for m in range(n_mt):
                nc.tensor.matmul(yp[:], A[:, m, t * P:(t + 1) * P], wdr[:, m],
                                 start=(m == 0), stop=(m == n_mt - 1))
            ysb = iopool.tile([P, HC], F32, tag="ysb")
            if rws is None:
                nc.scalar.copy(ysb[:], yp[:])
            else:
                ti = rw_base + t
                nc.scalar.mul(ysb[:], yp[:], rws[:, ti:ti + 1])
            nc.sync.dma_start(
                y_dram[toff + t * P: toff + (t + 1) * P, hc * HC:(hc + 1) * HC],
                ysb[:])


def build_bass(slot_caps):
    ncap = sum(slot_caps)
    nt = ncap // P
    nc = bacc_mod.Bacc(trn_type="TRN2")

    xt = nc.dram_tensor("xt", [H, ncap], F32R, kind="ExternalInput")
    rw = nc.dram_tensor("rw", [P, nt], F32, kind="ExternalInput")
    wg = nc.dram_tensor("wg", [NSLOTS, NM, P, H], F32R, kind="ExternalInput")
    wu = nc.dram_tensor("wu", [NSLOTS, NM, P, H], F32R, kind="ExternalInput")
    wd = nc.dram_tensor("wd", [NSLOTS, NHC, P, NM * HC], F32R, kind="ExternalInput")
    xts = nc.dram_tensor("xts", [H, TOK_SH], F32R, kind="ExternalInput")
    swg = nc.dram_tensor("swg", [NMS, P, H], F32R, kind="ExternalInput")
    swu = nc.dram_tensor("swu", [NMS, P, H], F32R, kind="ExternalInput")
    swd = nc.dram_tensor("swd", [NHC, P, NMS * HC], F32R, kind="ExternalInput")
    y = nc.dram_tensor("y", [ncap, H], F32, kind="ExternalOutput")
    ys = nc.dram_tensor("ys", [TOK_SH, H], F32, kind="ExternalOutput")

    with tile.TileContext(nc) as tc:
        with tc.tile_pool(name="rwp", bufs=1) as rwp:
            rws = rwp.tile([P, nt], F32)
            nc.sync.dma_start(rws[:], rw[:])

            # ---- expert phase ----
            with tc.tile_pool(name="eres", bufs=1) as eres, \
                 tc.tile_pool(name="ew", bufs=2) as ew, \
                 tc.tile_pool(name="eio", bufs=3) as eio, \
                 tc.tile_pool(name="eps", bufs=2, space="PSUM") as eps:
                soff = 0
                for s in range(NSLOTS):
                    coff = 0
                    for ck in _chunks_of(slot_caps[s]):
                        toff = soff + coff
                        _mlp_chunk(nc, ew, eio, eres, eps, xt, toff, ck,
                                   wg[s], wu[s], wd[s], NM,
                                   rws, toff // P, y)
                        coff += ck
                    soff += slot_caps[s]

            # ---- shared-expert phase ----
            with tc.tile_pool(name="sres", bufs=1) as sres, \
                 tc.tile_pool(name="sw", bufs=1) as swp, \
                 tc.tile_pool(name="sio", bufs=3) as sio, \
                 tc.tile_pool(name="sps", bufs=2, space="PSUM") as sps:
                for ci in range(TOK_SH // 512):
                    _mlp_chunk(nc, swp, sio, sres, sps, xts, ci * 512, 512,
                               swg, swu, swd, NMS, None, 0, ys)
    nc.finalize()
    return nc


def _route(x, gate_w):
    """Replicate the reference routing in numpy fp32."""
    logits = x @ gate_w                                   # [T, E]
    scores = 1.0 / (1.0 + np.exp(-logits))
    sg = scores.reshape(T, N_GROUPS, E // N_GROUPS)
    group_scores = sg.max(axis=-1)
    top_groups = np.argsort(-group_scores, axis=1, kind="stable")[:, :N_LIMITED]
    mask = np.ones((T, N_GROUPS), dtype=bool)
    mask[np.arange(T)[:, None], top_groups] = False
    sgm = np.where(mask[:, :, None], -np.inf, sg).reshape(T, E)
    sel = np.argsort(-sgm, axis=1, kind="stable")[:, :TOPK]     # [T, K]
    w = np.take_along_axis(scores, sel, axis=1)
    w = w / w.sum(axis=1, keepdims=True)
    return sel.astype(np.int64), w.astype(np.float32)


def kernel(hidden_states, gate_w, w_gate, w_up, w_down, sw_gate, sw_up, sw_down):
    x = np.ascontiguousarray(np.asarray(hidden_states, dtype=np.float32).reshape(T, H))
    gate_w = np.asarray(gate_w, dtype=np.float32)
    w_gate = np.asarray(w_gate, dtype=np.float32)
    w_up = np.asarray(w_up, dtype=np.float32)
    w_down = np.asarray(w_down, dtype=np.float32)
    sw_gate = np.asarray(sw_gate, dtype=np.float32)
    sw_up = np.asarray(sw_up, dtype=np.float32)
    sw_down = np.asarray(sw_down, dtype=np.float32)

    # ---- 1. routing ----
    sel, wts = _route(x, gate_w)
    sel_flat = sel.ravel()                       # pair index -> expert
    counts = np.bincount(sel_flat, minlength=E)

    # ---- 2. expert -> (core, slot) assignment ----
    order = np.argsort(-counts, kind="stable")   # experts by count desc
    slot_caps = []
    assign = np.empty((NCORES, NSLOTS), dtype=np.int64)
    for s in range(NSLOTS):
        grp = order[s * NCORES:(s + 1) * NCORES]
        assign[:, s] = grp
        slot_caps.append(max(P, _round_up(int(counts[grp].max()), P)))
    ncap = sum(slot_caps)
    soffs = np.cumsum([0] + slot_caps)[:-1]

    # token-pair rows per expert, ascending pair index (stable)
    rows_of = [np.flatnonzero(sel_flat == e) for e in range(E)]

    # ---- 3. per-core inputs ----
    # shared tensors (identical on every core)
    swg_t = np.ascontiguousarray(
        sw_gate.reshape(NH, P, NMS, P).transpose(2, 1, 0, 3).reshape(NMS, P, H))
    swu_t = np.ascontiguousarray(
        sw_up.reshape(NH, P, NMS, P).transpose(2, 1, 0, 3).reshape(NMS, P, H))
    swd_t = np.ascontiguousarray(
        sw_down.reshape(NMS, P, NHC, HC).transpose(2, 1, 0, 3).reshape(NHC, P, NMS * HC))

    in_maps = []
    for c in range(NCORES):
        el = assign[c]                            # 4 expert ids
        xt_c = np.zeros((ncap, H), dtype=np.float32)
        rw_c = np.zeros(ncap, dtype=np.float32)
        for s in range(NSLOTS):
            e = el[s]
            r = rows_of[e]
            n = len(r)
            xt_c[soffs[s]:soffs[s] + n] = x[r // TOPK]
            rw_c[soffs[s]:soffs[s] + n] = wts[r // TOPK, r % TOPK]
        xt_c = np.ascontiguousarray(xt_c.T)       # [H, ncap]
        rw_t = np.ascontiguousarray(rw_c.reshape(ncap // P, P).T)   # [128, nt]

        wg_c = np.ascontiguousarray(
            w_gate[el].reshape(NSLOTS, NH, P, NM, P)
            .transpose(0, 3, 2, 1, 4).reshape(NSLOTS, NM, P, H))
        wu_c = np.ascontiguousarray(
            w_up[el].reshape(NSLOTS, NH, P, NM, P)
            .transpose(0, 3, 2, 1, 4).reshape(NSLOTS, NM, P, H))
        wd_c = np.ascontiguousarray(
            w_down[el].reshape(NSLOTS, NM, P, NHC, HC)
            .transpose(0, 3, 2, 1, 4).reshape(NSLOTS, NHC, P, NM * HC))

        xts_c = np.ascontiguousarray(x[c * TOK_SH:(c + 1) * TOK_SH].T)  # [H, 1024]

        in_maps.append({
            "xt": xt_c, "rw": rw_t,
            "wg": wg_c, "wu": wu_c, "wd": wd_c,
            "xts": xts_c, "swg": swg_t, "swu": swu_t, "swd": swd_t,
        })

    # ---- 4. build + run on 8 cores ----
    nc = build_bass(slot_caps)
    global LAST_NC, LAST_RESULTS
    LAST_NC = nc
    res = run_bass_kernel_spmd(nc, in_maps, core_ids=list(range(NCORES)))
    LAST_RESULTS = res
    if res.exec_time_ns is not None:
        print(f"HW exec time: {res.exec_time_ns} ns")

    # ---- 5. combine on host ----
    d_pairs = np.empty((T * TOPK, H), dtype=np.float32)
    for c in range(NCORES):
        y_c = res.results[c]["y"]
        for s in range(NSLOTS):
            r = rows_of[assign[c, s]]
            d_pairs[r] = y_c[soffs[s]:soffs[s] + len(r)]
    expert_out = d_pairs.reshape(T, TOPK, H).sum(axis=1)

    shared_out = np.concatenate([res.results[c]["ys"] for c in range(NCORES)], axis=0)

    out = (expert_out + shared_out).reshape(B, S, H).astype(np.float32)
    return out


# revision 8
# speedup vs baseline: 1.1833x; 1.1833x over previous
"""DeepseekV3 MoE layer on 8 Trainium2 NeuronCores (expert-parallel).

Contract: kernel(**inputs) takes the FULL unsharded inputs and returns the
FULL output [4, 2048, 2048] f32.

Strategy:
  - Routing (sigmoid gate + group-limited top-6) computed on host in numpy.
  - Expert parallelism: 32 experts -> 8 cores x 4 slots. Experts are
    assigned to (core, slot) by sorted token count so every core runs an
    identical static program (slot capacities shared across cores).
  - Token dispatch on host: tokens gathered/padded per expert, transposed
    to feature-major [H, ncap] blocks; per-token routing weights applied
    on device; host scatter-adds the (disjoint) results.
  - Shared MLP data-parallel over tokens (1024 tokens per core).
  - Device kernel: fp32r matmuls (full PE rate, ~1.5e-4 matmul error),
    silu/mul on ACT/DVE, weights streamed from HBM once per token chunk.
"""
import sys
import os

sys.path.insert(0, "/opt/trn_rl_repo")

import numpy as np

import concourse.bacc as bacc_mod
import concourse.mybir as mybir
import concourse.tile as tile
from concourse.bass_utils import run_bass_kernel_spmd

F32 = mybir.dt.float32
F32R = mybir.dt.float32r
P = 128

# Problem constants (hardcoded per contract)
B, S, H = 4, 2048, 2048
T = B * S                      # 8192 tokens
E = 32                         # experts
TOPK = 6
N_GROUPS = 4
N_LIMITED = 2
MI = 1408                      # expert intermediate
SH = 2816                      # shared intermediate
NH = H // P                    # 16 h-tiles
NM = MI // P                   # 11 m-tiles (expert)
NMS = SH // P                  # 22 m-tiles (shared)
NCORES = 8
NSLOTS = 4
HC = 512                       # stage-2 output column chunk
NHC = H // HC                  # 4
TOK_SH = T // NCORES           # 1024 shared-MLP tokens per core


def _round_up(x, m):
    return ((x + m - 1) // m) * m


_GOOD_CHUNKS = (896, 768, 512, 384, 256)  # ck % 512 in {0, 256, 384}: no <256 pieces


def _chunks_of(cap):
    """Split a slot capacity (multiple of 128) into the fewest chunks from
    _GOOD_CHUNKS (fp32r needs moving pieces >=256 for full PE rate)."""
    best = {0: []}
    for c in range(128, cap + 1, 128):
        cands = []
        for g in _GOOD_CHUNKS:
            if g <= c and (c - g) in best:
                cands.append([g] + best[c - g])
        if c in (128,) or not cands:
            if (c - 128) in best:
                cands.append([128] + best[c - 128])   # last-resort tiny chunk
        if cands:
            best[c] = min(cands, key=lambda l: (len(l), l.count(128)))
    return sorted(best[cap], reverse=True)


def _pieces_of(ck):
    """Split a chunk into moving-dim pieces of <=512 (bank-aligned)."""
    out = []
    off = 0
    while off < ck:
        pl = min(512, ck - off)
        out.append((off, pl))
        off += pl
    return out


def _mlp_chunk(nc, wpool, iopool, respool, pspool, xt_dram, toff, ck,
               wg_dram, wu_dram, wd_dram, n_mt, rws, rw_base, y_dram):
    """One token chunk through gate/up/silu*up/down (+ routing-weight scale).

    xt_dram: [H, ncap-like] feature-major tokens (f32r dram)
    wg_dram/wu_dram: [n_mt, 128, H]   (m-tile, p, ko*128)
    wd_dram: [NHC, 128, n_mt*512]     (hc, p(m-row), mt*512)
    rws: resident [128, NT] routing-weight tile or None
    y_dram: [ntok, H] output rows
    """
    ACT = mybir.ActivationFunctionType
    # ---- load + round X^T chunk (per h-tile so PE can start early) ----
    xtr = respool.tile([P, NH, ck], F32R, tag="xtr")
    for h in range(NH):
        nc.sync.dma_start(xtr[:, h], xt_dram[h * P:(h + 1) * P, toff:toff + ck])
        nc.vector.tensor_copy(out=xtr[:, h], in_=xtr[:, h])

    A = respool.tile([P, n_mt, ck], F32R, tag="A")
    pieces = _pieces_of(ck)
    # ---- stage 1: G = X@Wg, U = X@Wu, A = silu(G)*U  (feature-major) ----
    for m in range(n_mt):
        wgr = wpool.tile([P, NH, P], F32R, tag="wgr")
        wur = wpool.tile([P, NH, P], F32R, tag="wur")
        nc.sync.dma_start(wgr[:], wg_dram[m].rearrange("p (ko x) -> p ko x", x=P))
        nc.sync.dma_start(wur[:], wu_dram[m].rearrange("p (ko x) -> p ko x", x=P))
        nc.vector.tensor_copy(out=wgr[:], in_=wgr[:])
        nc.vector.tensor_copy(out=wur[:], in_=wur[:])
        for (po, pl) in pieces:
            g = pspool.tile([P, pl], F32, tag="g")
            u = pspool.tile([P, pl], F32, tag="u")
            for h in range(NH):
                nc.tensor.matmul(g[:], wgr[:, h], xtr[:, h, po:po + pl],
                                 start=(h == 0), stop=(h == NH - 1))
            for h in range(NH):
                nc.tensor.matmul(u[:], wur[:, h], xtr[:, h, po:po + pl],
                                 start=(h == 0), stop=(h == NH - 1))
            sg = iopool.tile([P, pl], F32, tag="sg")
            nc.scalar.activation(out=sg[:], in_=g[:], func=ACT.Silu)
            # fp32r output = the rounding producer for stage 2
            nc.vector.tensor_mul(out=A[:, m, po:po + pl], in0=sg[:], in1=u[:])

    # ---- stage 2: Y = A @ Wd, scaled by routing weight, token-major ----
    ntt = ck // P
    for hc in range(NHC):
        wdr = wpool.tile([P, n_mt, HC], F32R, tag="wdr")
        nc.sync.dma_start(wdr[:], wd_dram[hc].rearrange("p (mt x) -> p mt x", x=HC))
        nc.vector.tensor_copy(out=wdr[:], in_=wdr[:])
        for t in range(ntt):
            yp = pspool.tile([P, HC], F32, tag="y")
            for m in range(n_mt):
                nc.tensor.matmul(yp[:], A[:, m, t * P:(t + 1) * P], wdr[:, m],
                                 start=(m == 0), stop=(m == n_mt - 1))
            ysb = iopool.tile([P, HC], F32, tag="ysb")
            if rws is None:
                nc.scalar.copy(ysb[:], yp[:])
            else:
                ti = rw_base + t
                nc.scalar.mul(ysb[:], yp[:], rws[:, ti:ti + 1])
            nc.sync.dma_start(
                y_dram[toff + t * P: toff + (t + 1) * P, hc * HC:(hc + 1) * HC],
                ysb[:])


def build_bass(slot_caps):
    ncap = sum(slot_caps)
    nt = ncap // P
    nc = bacc_mod.Bacc(trn_type="TRN2")

    xt = nc.dram_tensor("xt", [H, ncap], F32R, kind="ExternalInput")
    rw = nc.dram_tensor("rw", [P, nt], F32, kind="ExternalInput")
    wg = nc.dram_tensor("wg", [NSLOTS, NM, P, H], F32R, kind="ExternalInput")
    wu = nc.dram_tensor("wu", [NSLOTS, NM, P, H], F32R, kind="ExternalInput")
    wd = nc.dram_tensor("wd", [NSLOTS, NHC, P, NM * HC], F32R, kind="ExternalInput")
    xts = nc.dram_tensor("xts", [H, TOK_SH], F32R, kind="ExternalInput")
    swg = nc.dram_tensor("swg", [NMS, P, H], F32R, kind="ExternalInput")
    swu = nc.dram_tensor("swu", [NMS, P, H], F32R, kind="ExternalInput")
    swd = nc.dram_tensor("swd", [NHC, P, NMS * HC], F32R, kind="ExternalInput")
    y = nc.dram_tensor("y", [ncap, H], F32, kind="ExternalOutput")
    ys = nc.dram_tensor("ys", [TOK_SH, H], F32, kind="ExternalOutput")

    with tile.TileContext(nc) as tc:
        with tc.tile_pool(name="rwp", bufs=1) as rwp:
            rws = rwp.tile([P, nt], F32)
            nc.sync.dma_start(rws[:], rw[:])

            # ---- expert phase ----
            with tc.tile_pool(name="eres", bufs=1) as eres, \
                 tc.tile_pool(name="ew", bufs=2) as ew, \
                 tc.tile_pool(name="eio", bufs=3) as eio, \
                 tc.tile_pool(name="eps", bufs=2, space="PSUM") as eps:
                soff = 0
                for s in range(NSLOTS):
                    coff = 0
                    for ck in _chunks_of(slot_caps[s]):
                        toff = soff + coff
                        _mlp_chunk(nc, ew, eio, eres, eps, xt, toff, ck,
                                   wg[s], wu[s], wd[s], NM,
                                   rws, toff // P, y)
                        coff += ck
                    soff += slot_caps[s]

            # ---- shared-expert phase: one 1024-token chunk, weights
            # streamed exactly once; xtr pool closed before the down-proj
            # slices are allocated so everything fits in SBUF ----
            ACT = mybir.ActivationFunctionType
            with tc.tile_pool(name="sres", bufs=1) as sres, \
                 tc.tile_pool(name="sio", bufs=2) as sio, \
                 tc.tile_pool(name="sps", bufs=2, space="PSUM") as sps:
                A = sres.tile([P, NMS, TOK_SH], F32R)
                with tc.tile_pool(name="sx", bufs=1) as sxp, \
                     tc.tile_pool(name="s1w", bufs=2) as s1w:
                    xtr = sxp.tile([P, NH, TOK_SH], F32R)
                    for h in range(NH):
                        nc.sync.dma_start(xtr[:, h],
                                          xts[h * P:(h + 1) * P, :])
                        nc.vector.tensor_copy(out=xtr[:, h], in_=xtr[:, h])
                    for m in range(NMS):
                        wgr = s1w.tile([P, NH, P], F32R, tag="wgr")
                        wur = s1w.tile([P, NH, P], F32R, tag="wur")
                        nc.sync.dma_start(
                            wgr[:], swg[m].rearrange("p (ko x) -> p ko x", x=P))
                        nc.sync.dma_start(
                            wur[:], swu[m].rearrange("p (ko x) -> p ko x", x=P))
                        nc.vector.tensor_copy(out=wgr[:], in_=wgr[:])
                        nc.vector.tensor_copy(out=wur[:], in_=wur[:])
                        for (po, pl) in _pieces_of(TOK_SH):
                            g = sps.tile([P, pl], F32, tag="g")
                            u = sps.tile([P, pl], F32, tag="u")
                            for h in range(NH):
                                nc.tensor.matmul(g[:], wgr[:, h],
                                                 xtr[:, h, po:po + pl],
                                                 start=(h == 0), stop=(h == NH - 1))
                            for h in range(NH):
                                nc.tensor.matmul(u[:], wur[:, h],
                                                 xtr[:, h, po:po + pl],
                                                 start=(h == 0), stop=(h == NH - 1))
                            sg = sio.tile([P, pl], F32, tag="sg")
                            nc.scalar.activation(out=sg[:], in_=g[:], func=ACT.Silu)
                            nc.vector.tensor_mul(out=A[:, m, po:po + pl],
                                                 in0=sg[:], in1=u[:])
                with tc.tile_pool(name="s2w", bufs=2) as s2w:
                    for hc in range(NHC):
                        wdr = s2w.tile([P, NMS, HC], F32R, tag="wdr")
                        nc.sync.dma_start(
                            wdr[:], swd[hc].rearrange("p (mt x) -> p mt x", x=HC))
                        nc.vector.tensor_copy(out=wdr[:], in_=wdr[:])
                        for t in range(TOK_SH // P):
                            yp = sps.tile([P, HC], F32, tag="y")
                            for m in range(NMS):
                                nc.tensor.matmul(yp[:], A[:, m, t * P:(t + 1) * P],
                                                 wdr[:, m],
                                                 start=(m == 0), stop=(m == NMS - 1))
                            ysb = sio.tile([P, HC], F32, tag="ysb")
                            nc.scalar.copy(ysb[:], yp[:])
                            nc.sync.dma_start(
                                ys[t * P:(t + 1) * P, hc * HC:(hc + 1) * HC], ysb[:])
    nc.finalize()
    return nc


def _route(x, gate_w):
    """Replicate the reference routing in numpy fp32."""
    logits = x @ gate_w                                   # [T, E]
    scores = 1.0 / (1.0 + np.exp(-logits))
    sg = scores.reshape(T, N_GROUPS, E // N_GROUPS)
    group_scores = sg.max(axis=-1)
    top_groups = np.argsort(-group_scores, axis=1, kind="stable")[:, :N_LIMITED]
    mask = np.ones((T, N_GROUPS), dtype=bool)
    mask[np.arange(T)[:, None], top_groups] = False
    sgm = np.where(mask[:, :, None], -np.inf, sg).reshape(T, E)
    sel = np.argsort(-sgm, axis=1, kind="stable")[:, :TOPK]     # [T, K]
    w = np.take_along_axis(scores, sel, axis=1)
    w = w / w.sum(axis=1, keepdims=True)
    return sel.astype(np.int64), w.astype(np.float32)


def prepare(hidden_states, gate_w, w_gate, w_up, w_down, sw_gate, sw_up, sw_down):
    """Host-side routing + sharding. Returns (slot_caps, in_maps, meta)."""
    x = np.ascontiguousarray(np.asarray(hidden_states, dtype=np.float32).reshape(T, H))
    gate_w = np.asarray(gate_w, dtype=np.float32)
    w_gate = np.asarray(w_gate, dtype=np.float32)
    w_up = np.asarray(w_up, dtype=np.float32)
    w_down = np.asarray(w_down, dtype=np.float32)
    sw_gate = np.asarray(sw_gate, dtype=np.float32)
    sw_up = np.asarray(sw_up, dtype=np.float32)
    sw_down = np.asarray(sw_down, dtype=np.float32)

    # ---- 1. routing ----
    sel, wts = _route(x, gate_w)
    sel_flat = sel.ravel()                       # pair index -> expert
    counts = np.bincount(sel_flat, minlength=E)

    # ---- 2. expert -> (core, slot) assignment ----
    order = np.argsort(-counts, kind="stable")   # experts by count desc
    slot_caps = []
    assign = np.empty((NCORES, NSLOTS), dtype=np.int64)
    for s in range(NSLOTS):
        grp = order[s * NCORES:(s + 1) * NCORES]
        assign[:, s] = grp
        slot_caps.append(max(P, _round_up(int(counts[grp].max()), P)))
    ncap = sum(slot_caps)
    soffs = np.cumsum([0] + slot_caps)[:-1]

    # token-pair rows per expert, ascending pair index (stable)
    rows_of = [np.flatnonzero(sel_flat == e) for e in range(E)]

    # ---- 3. per-core inputs ----
    # shared tensors (identical on every core)
    swg_t = np.ascontiguousarray(
        sw_gate.reshape(NH, P, NMS, P).transpose(2, 1, 0, 3).reshape(NMS, P, H))
    swu_t = np.ascontiguousarray(
        sw_up.reshape(NH, P, NMS, P).transpose(2, 1, 0, 3).reshape(NMS, P, H))
    swd_t = np.ascontiguousarray(
        sw_down.reshape(NMS, P, NHC, HC).transpose(2, 1, 0, 3).reshape(NHC, P, NMS * HC))

    in_maps = []
    for c in range(NCORES):
        el = assign[c]                            # 4 expert ids
        xt_c = np.zeros((ncap, H), dtype=np.float32)
        rw_c = np.zeros(ncap, dtype=np.float32)
        for s in range(NSLOTS):
            e = el[s]
            r = rows_of[e]
            n = len(r)
            xt_c[soffs[s]:soffs[s] + n] = x[r // TOPK]
            rw_c[soffs[s]:soffs[s] + n] = wts[r // TOPK, r % TOPK]
        xt_c = np.ascontiguousarray(xt_c.T)       # [H, ncap]
        rw_t = np.ascontiguousarray(rw_c.reshape(ncap // P, P).T)   # [128, nt]

        wg_c = np.ascontiguousarray(
            w_gate[el].reshape(NSLOTS, NH, P, NM, P)
            .transpose(0, 3, 2, 1, 4).reshape(NSLOTS, NM, P, H))
        wu_c = np.ascontiguousarray(
            w_up[el].reshape(NSLOTS, NH, P, NM, P)
            .transpose(0, 3, 2, 1, 4).reshape(NSLOTS, NM, P, H))
        wd_c = np.ascontiguousarray(
            w_down[el].reshape(NSLOTS, NM, P, NHC, HC)
            .transpose(0, 3, 2, 1, 4).reshape(NSLOTS, NHC, P, NM * HC))

        xts_c = np.ascontiguousarray(x[c * TOK_SH:(c + 1) * TOK_SH].T)  # [H, 1024]

        in_maps.append({
            "xt": xt_c, "rw": rw_t,
            "wg": wg_c, "wu": wu_c, "wd": wd_c,
            "xts": xts_c, "swg": swg_t, "swu": swu_t, "swd": swd_t,
        })

    meta = {"rows_of": rows_of, "assign": assign, "soffs": soffs}
    return slot_caps, in_maps, meta


def combine(results, meta):
    """Host-side unshard: scatter expert outputs back + add shared."""
    rows_of, assign, soffs = meta["rows_of"], meta["assign"], meta["soffs"]
    d_pairs = np.empty((T * TOPK, H), dtype=np.float32)
    for c in range(NCORES):
        y_c = results[c]["y"]
        for s in range(NSLOTS):
            r = rows_of[assign[c, s]]
            d_pairs[r] = y_c[soffs[s]:soffs[s] + len(r)]
    expert_out = d_pairs.reshape(T, TOPK, H).sum(axis=1)
    shared_out = np.concatenate([results[c]["ys"] for c in range(NCORES)], axis=0)
    return (expert_out + shared_out).reshape(B, S, H).astype(np.float32)


def kernel(hidden_states, gate_w, w_gate, w_up, w_down, sw_gate, sw_up, sw_down):
    slot_caps, in_maps, meta = prepare(hidden_states, gate_w, w_gate, w_up,
                                       w_down, sw_gate, sw_up, sw_down)
    nc = build_bass(slot_caps)
    global LAST_NC, LAST_RESULTS
    LAST_NC = nc
    res = run_bass_kernel_spmd(nc, in_maps, core_ids=list(range(NCORES)))
    LAST_RESULTS = res
    if res.exec_time_ns is not None:
        print(f"HW exec time: {res.exec_time_ns} ns")
    return combine(res.results, meta)
